# revision 11
# baseline (speedup 1.0000x reference)
"""Gumbel-softmax VQ discretization kernel for Trainium2 (8 NeuronCores).

Reference computation (per flattened latent r of N=262144, codebook K=256):
    dist[r,k] = |z_r - cb_k|
    s[r,k]    = (gumbel[r,k] - dist[r,k]) / tau
    p[r,:]    = softmax(s[r,:])
    disc[r]   = sum_k p[r,k] * cb_k
    idx[r]    = argmin_k dist[r,k]
    avg[k]    = mean_r p[r,k];  perplexity = exp(-sum avg*log(avg+1e-10))

Sharding: pure data-parallel over rows, 32768 rows/core on 8 cores.
Dominant traffic: gumbel (256 MB f32) -> memory-bound.

Engine split per [128 rows x 256 k] tile (8 tiles per 1 MB DMA block):
    VEC    : dist = abs_max(cb - z, 0)        (tensor_scalar, 2 ops fused)
    GPSIMD : s'   = g - dist                  (scalar_tensor_tensor fused)
    ACT    : e    = exp(inv_tau * s'), den=row-sum   (activation accum_out)
    VEC    : num  = row-sum(e * cb)           (tensor_tensor_reduce)
    PE     : avg_psum += rden^T @ e           (PSUM accumulation)
"""

import sys
import os

for _p in ("/opt/trn_rl_repo",):
    if _p not in sys.path:
        sys.path.insert(0, _p)

import numpy as np

B, NW, D, K = 4, 16, 4096, 256
N = B * NW * D              # 262144 rows
NCORES = 8
NC_ROWS = N // NCORES       # 32768 rows per core
P = 128                     # partitions
TILES = NC_ROWS // P        # 256 tiles of [128, 256] per core
BLK_TILES = 8               # tiles per DMA block (1 MB)
BLOCKS = TILES // BLK_TILES # 32 blocks

_cached = {}


def _patch_tile_drain():
    """The walrus build in this container rejects instructions carrying more
    than one semaphore sync-wait. Tile's kernel-tail drain waits on every
    active proc's semaphore at once; split those waits across single-wait
    sync-engine nops instead."""
    import concourse.tile as tile
    from concourse.vector_clock import ScopedClock

    if getattr(tile.TileContext, "_drain_patched", False):
        return

    def _drain_and_barrier(self, tick_clock, wait_clock):
        nc = self.nc
        drain_inst = nc.sync.drain()
        wait_clock.add_sem_waits(
            drain_inst.ins, ScopedClock({None: tick_clock.global_clock})
        )
        si = drain_inst.ins.sync_info
        waits = list(si.on_wait or [])
        if len(waits) > 1:
            si.on_wait = waits[:1]
            for w in waits[1:]:
                nop_bi = nc.sync.nop(nofuse=True, hint="drain_split")
                nsi = nop_bi.ins.sync_info
                if nsi is None:
                    import concourse.mybir as mybir
                    nop_bi.ins.sync_info = mybir.SyncInfo(
                        on_wait=[w], on_update=[])
                else:
                    nsi.on_wait = [w]

        nc.all_engine_barrier()
        assert self.sems is not None
        popped = nc._tile_sem_poison_stack.pop()
        assert popped is self._sem_poison
        nc.clear_and_free_semaphores(list(self.sems.allocated().values()))
        nc.all_engine_barrier()

    tile.TileContext._drain_and_barrier = _drain_and_barrier
    tile.TileContext._drain_patched = True


def _split_multi_waits(nc):
    """Walrus in this container allows at most one semaphore sync-wait per
    instruction. Hoist extra waits onto same-engine nops placed just before
    the over-constrained instruction (identical sync semantics: the engine
    executes nop-waits first, then the real op)."""
    import concourse.mybir as mybir

    fn = nc.m.functions[0]

    def _detach_last(nop_ins):
        for bb2 in fn.blocks:
            lst = bb2.instructions
            if lst and lst[-1] is nop_ins:
                lst.pop()
                return
        raise RuntimeError("freshly created nop not found at any block tail")

    for bb in list(fn.blocks):
        out = []
        for inst in bb.instructions:
            si = getattr(inst, "sync_info", None)
            waits = list(si.on_wait) if (si and si.on_wait) else []
            if len(waits) > 1:
                eng = inst.engine
                for w in waits[:-1]:
                    nb = nc.engines[eng].nop(nofuse=True, hint="wait_split")
                    nop_ins = nb.ins
                    _detach_last(nop_ins)
                    nop_ins.sync_info = mybir.SyncInfo(
                        on_wait=[w], on_update=[])
                    out.append(nop_ins)
                si.on_wait = [waits[-1]]
            out.append(inst)
        bb.instructions[:] = out


def _build(inv_tau: float, reps: int = 1):
    import concourse.bass as bass
    import concourse.tile as tile
    import concourse.mybir as mybir

    _patch_tile_drain()

    f32 = mybir.dt.float32
    i32 = mybir.dt.int32
    Alu = mybir.AluOpType
    Act = mybir.ActivationFunctionType

    nc = bass.Bass()

    g_d = nc.declare_dram_parameter("g", [NC_ROWS, K], f32, isOutput=False)
    # consts = concat(cbb [P,K], zcols [P,TILES]) along free dim: one DMA,
    # one DMA-lane semaphore (TensorScalarPtr has a single sync-wait slot).
    c_d = nc.declare_dram_parameter("consts", [P, K + TILES], f32,
                                    isOutput=False)
    disc_d = nc.declare_dram_parameter("disc", [P, TILES], f32, isOutput=True)
    idx_d = nc.declare_dram_parameter("idx", [P, TILES], i32, isOutput=True)
    avg_d = nc.declare_dram_parameter("avgp", [1, K], f32, isOutput=True)

    with tile.TileContext(nc) as tc:
        with (
            tc.tile_pool(name="const", bufs=1) as constp,
            tc.tile_pool(name="gp", bufs=3) as gp,
            tc.tile_pool(name="ep", bufs=3) as ep,
            tc.tile_pool(name="scr", bufs=4) as scr,
            tc.tile_pool(name="stat", bufs=1) as statp,
            tc.tile_pool(name="psum", bufs=1, space="PSUM") as psp,
        ):
            csb = constp.tile([P, K + TILES], f32)
            nc.sync.dma_start(out=csb[:, :], in_=c_d[:, :])
            negz = constp.tile([P, TILES], f32)
            nc.vector.tensor_scalar(
                out=negz[:, :], in0=csb[:, K:K + TILES],
                scalar1=-1.0, scalar2=None, op0=Alu.mult)

            den = statp.tile([P, TILES], f32)
            num = statp.tile([P, TILES], f32)
            rden = statp.tile([P, TILES], f32)
            disc = statp.tile([P, TILES], f32)
            idxf = statp.tile([P, TILES], f32)
            idxi = statp.tile([P, TILES], i32)
            avg_sb = statp.tile([1, K], f32)

            avg_ps = psp.tile([1, K], f32)

            for rep in range(reps):
              for blk in range(BLOCKS):
                r0 = blk * BLK_TILES * P
                g_ap = g_d[r0:r0 + BLK_TILES * P, :].rearrange(
                    "(a p) k -> p a k", p=P)
                gt = gp.tile([P, BLK_TILES, K], f32)
                nc.sync.dma_start(out=gt[:, :, :], in_=g_ap)

                et = ep.tile([P, BLK_TILES, K], f32)

                for a in range(BLK_TILES):
                    t = blk * BLK_TILES + a
                    dist = scr.tile([P, K], f32, tag="dist")
                    # dist = |cb - z|  (ACT Abs with per-partition bias -z)
                    nc.scalar.activation(
                        out=dist[:, :], in_=csb[:, 0:K], func=Act.Abs,
                        bias=negz[:, t:t + 1], scale=1.0)
                    # s' = g - dist
                    sp = scr.tile([P, K], f32, tag="sp")
                    nc.vector.tensor_tensor(
                        out=sp[:, :], in0=gt[:, a, :], in1=dist[:, :],
                        op=Alu.subtract)
                    # e = exp(inv_tau * s'), den = row-sum(e)
                    nc.scalar.activation(
                        out=et[:, a, :], in_=sp[:, :], func=Act.Exp,
                        scale=float(inv_tau),
                        accum_out=den[:, t:t + 1])
                    # num = row-sum(e * cb)   (stt: out=(e*1)*cb, accum=sum)
                    ecb = scr.tile([P, K], f32, tag="ecb")
                    nc.vector.scalar_tensor_tensor(
                        out=ecb[:, :], in0=et[:, a, :], scalar=1.0,
                        in1=csb[:, 0:K],
                        op0=Alu.mult, op1=Alu.mult,
                        accum_out=num[:, t:t + 1])

                t0 = blk * BLK_TILES
                nc.vector.reciprocal(
                    rden[:, t0:t0 + BLK_TILES], den[:, t0:t0 + BLK_TILES])
                # disc = num * rden for the block
                nc.vector.tensor_tensor(
                    out=disc[:, t0:t0 + BLK_TILES],
                    in0=num[:, t0:t0 + BLK_TILES],
                    in1=rden[:, t0:t0 + BLK_TILES], op=Alu.mult)
                # avg_psum += rden_t^T @ e_t  for each tile in block
                for a in range(BLK_TILES):
                    t = blk * BLK_TILES + a
                    nc.tensor.matmul(
                        avg_ps[:, :], rden[:, t:t + 1], et[:, a, :],
                        start=(rep == 0 and t == 0),
                        stop=(rep == reps - 1 and t == TILES - 1))

            # indices: idx0 = clip((z+1)*127.5 + 0.5, 0, 255.49) -> int
            nc.vector.tensor_scalar(
                out=idxf[:, :], in0=csb[:, K:K + TILES],
                scalar1=127.5, scalar2=128.0, op0=Alu.mult, op1=Alu.add)
            nc.vector.tensor_scalar(
                out=idxf[:, :], in0=idxf[:, :],
                scalar1=0.0, scalar2=255.49, op0=Alu.max, op1=Alu.min)
            nc.vector.tensor_copy(idxi[:, :], idxf[:, :])

            nc.scalar.copy(avg_sb[:, :], avg_ps[:, :])

            nc.sync.dma_start(out=disc_d[:, :], in_=disc[:, :])
            nc.sync.dma_start(out=idx_d[:, :], in_=idxi[:, :])
            nc.sync.dma_start(out=avg_d[:, :], in_=avg_sb[:, :])

    _split_multi_waits(nc)
    return nc


def kernel(z, codebook, log_temperature, gumbel, _trace=False):
    from concourse.bass_utils import run_bass_kernel_spmd

    z = np.asarray(z, dtype=np.float32)
    codebook = np.asarray(codebook, dtype=np.float32)
    log_temperature = np.asarray(log_temperature, dtype=np.float32)
    gumbel = np.asarray(gumbel, dtype=np.float32)

    tau = float(np.exp(log_temperature.astype(np.float64))[0])
    inv_tau = 1.0 / tau

    key = round(inv_tau, 12)
    if key not in _cached:
        _cached[key] = _build(inv_tau)
    nc = _cached[key]

    z_flat = z.reshape(-1)
    cbb = np.ascontiguousarray(np.broadcast_to(codebook[None, :], (P, K)))

    in_maps = []
    for c in range(NCORES):
        zc = z_flat[c * NC_ROWS:(c + 1) * NC_ROWS]
        zcols = np.ascontiguousarray(zc.reshape(TILES, P).T)  # [P, TILES]
        gc = np.ascontiguousarray(gumbel[c * NC_ROWS:(c + 1) * NC_ROWS, :])
        consts = np.ascontiguousarray(
            np.concatenate([cbb, zcols], axis=1))
        in_maps.append({"g": gc, "consts": consts})

    res = run_bass_kernel_spmd(nc, in_maps, list(range(NCORES)),
                               trace=_trace)
    results = res.results

    disc_parts, idx_parts, avg_sum = [], [], np.zeros(K, dtype=np.float64)
    for c in range(NCORES):
        rc = results[c]
        disc_parts.append(np.asarray(rc["disc"]).T.reshape(-1))
        idx_parts.append(np.asarray(rc["idx"]).T.reshape(-1))
        avg_sum += np.asarray(rc["avgp"]).reshape(-1).astype(np.float64)

    disc = np.concatenate(disc_parts).astype(np.float32).reshape(B, NW, D)
    idx0 = np.concatenate(idx_parts).astype(np.int64)

    # Exact argmin refinement (ties/float boundaries): candidate idx0 is
    # within +-1 of the true argmin; replicate f32 |z-cb| comparisons with
    # first-min tie rule.
    cand = np.stack([np.clip(idx0 - 1, 0, K - 1),
                     np.clip(idx0, 0, K - 1),
                     np.clip(idx0 + 1, 0, K - 1)], axis=1)  # [N,3]
    dcand = np.abs(z_flat[:, None] - codebook[cand]).astype(np.float32)
    order = np.argsort(cand, axis=1)  # ascending k for first-min tie rule
    cand_sorted = np.take_along_axis(cand, order, axis=1)
    d_sorted = np.take_along_axis(dcand, order, axis=1)
    best = np.argmin(d_sorted, axis=1)  # np.argmin takes first minimum
    idx = cand_sorted[np.arange(N), best].astype(np.int32)

    avg_probs = (avg_sum / N).astype(np.float32)
    perplexity = np.exp(-np.sum(avg_probs * np.log(avg_probs + 1e-10),
                                dtype=np.float32)).astype(np.float32)

    if _trace:
        kernel.last_result = res
    return disc, np.float32(perplexity), idx
